# revision 13
# baseline (speedup 1.0000x reference)
"""AnchorOnlyMixtureRNN — 8-core Trainium2 kernel.

Architecture (scatter_memory): the model is two sequential scans plus dense
ops. The dominant cost — the 1024-step anchor-value (av) gated-LayerNorm
recurrence over state [B, A=64, D=512] — runs on the 8 NeuronCores, batch-
sharded 4 batches/core (pure data parallelism per the scan's per-batch
state). The cheap/dense host-friendly parts (embedding gather, z-trajectory,
batched gate GEMMs, 256-step decoder, vocab logits) run on host.

Device kernel per core (B_local=4):
  chains (b, a) -> tile s = b//2, partition p = a + 64*(b%2)
  - Z ships int8 (scale 1/16; dequant folded into the E selection matmul),
    staged from DRAM in 64-step blocks via SWDGE cast-DMA to [4, 64, 512].
  - per step: zb_s = E_s.T @ Zstage[:, j, :] on TensorE (PSUM);
    gated blend + LayerNorm on VectorE/ScalarE with the normalize folded
    into the next step's decay: x_t = (x_{t-1} - m)·(omg_t/stde) + g_t·zb_t.
  - final normalize + DMA av out.
"""
import math
import numpy as np

D = 512
A = 64
V_OUT = 32000
B = 32
S_ENC = 1024
S_DEC = 256
EPS = 1e-6
N_CORES = 8
BL = B // N_CORES          # 4 local batches per core
TBLK = 64                  # z staging block (steps)
QS = 16.0                  # int8 quant scale for Z

_CACHE = {}


def _ln(x, g, b):
    m = x.mean(axis=-1, keepdims=True)
    s = x.std(axis=-1, ddof=1, keepdims=True)
    return g * (x - m) / (s + EPS) + b


def _sigmoid(x):
    return 1.0 / (1.0 + np.exp(-x))


# ---------------------------------------------------------------- Bass ----
def _build_phase3_nc(S=S_ENC):
    import concourse.bacc as bacc
    import concourse.tile as tile
    from concourse import mybir

    f32 = mybir.dt.float32
    bf16 = mybir.dt.bfloat16
    i8 = mybir.dt.int8
    Alu = mybir.AluOpType
    Act = mybir.ActivationFunctionType

    nc = bacc.Bacc("TRN2", target_bir_lowering=False)
    Z_d = nc.declare_dram_parameter("Z", [BL, S, D], i8, isOutput=False)
    G_d = nc.declare_dram_parameter("G", [128, 2, S], bf16, isOutput=False)
    E_d = nc.declare_dram_parameter("E", [BL, 2, 128], bf16, isOutput=False)
    out_d = nc.declare_dram_parameter("avout", [2, 128, D], f32, isOutput=True)

    with tile.TileContext(nc) as tc:
        with (
            tc.tile_pool(name="big", bufs=1) as big,
            tc.tile_pool(name="stage", bufs=2) as stg,
            tc.tile_pool(name="work", bufs=2) as work,
            tc.tile_pool(name="st", bufs=2) as stp,
            tc.tile_pool(name="ps", bufs=4, space="PSUM") as ps,
        ):
            Gbf = big.tile([128, 2, S], bf16, tag="Gbf")
            nc.sync.dma_start(Gbf[:], G_d[:])
            Et = big.tile([BL, 2, 128], bf16, tag="E")
            nc.sync.dma_start(Et[:], E_d[:])
            # cast gates to f32 (scalar-AP operands) and compute 1-g
            Gt = big.tile([128, 2, S], f32, tag="G")
            nc.vector.tensor_copy(Gt[:], Gbf[:])
            OMGt = big.tile([128, 2, S], f32, tag="OMG")
            nc.vector.tensor_scalar(OMGt[:], Gt[:], -1.0, 1.0, Alu.mult,
                                    Alu.add)

            # persistent state: unnormalized x per tile
            x = [big.tile([128, D], f32, name=f"xs{s}", tag=f"xs{s}")
                 for s in range(2)]
            # per-step stat tiles (rotate via pool)
            mean_p = None
            rstd_p = None
            romg_p = None

            nblk = (S + TBLK - 1) // TBLK
            for blk in range(nblk):
                t0b = blk * TBLK
                nstep = min(TBLK, S - t0b)
                zst = stg.tile([BL, TBLK, D], bf16, tag="zst")
                nc.gpsimd.dma_start(zst[:, :nstep, :],
                                    Z_d[:, t0b:t0b + nstep, :])
                for j in range(nstep):
                    t = t0b + j
                    # sum(x) == sum(t1): the y term is centered (sum 0)
                    s1 = stp.tile([128, 2], f32, tag="s1")
                    s2 = stp.tile([128, 2], f32, tag="s2")
                    for s in range(2):
                        zbs = ps.tile([128, D], f32, name=f"zb{s}",
                                      tag=f"zb{s}")
                        nc.tensor.matmul(zbs[:], Et[:, s, :], zst[:, j, :],
                                         start=True, stop=True)
                        # t1 = zb * g (DVE, PSUM read), accum -> s1
                        t1 = work.tile([128, D], f32, name=f"t1{s}",
                                       tag=f"t1{s}")
                        nc.vector.tensor_scalar(t1[:], zbs[:],
                                                Gt[:, s, t:t + 1], 0.0,
                                                Alu.mult, Alu.add,
                                                accum_out=s1[:, s:s + 1])
                        if t == 0:
                            nc.vector.tensor_copy(x[s][:], t1[:])
                        else:
                            # y = (x_prev - m)·romg ; x = y + t1
                            y = work.tile([128, D], f32, name=f"y{s}",
                                          tag=f"y{s}")
                            nc.vector.tensor_scalar(
                                y[:], x[s][:], mean_p[:, s:s + 1],
                                romg_p[:, s:s + 1], Alu.subtract, Alu.mult)
                            nc.vector.tensor_tensor(x[s][:], y[:], t1[:],
                                                    Alu.add)
                        # xsq (dummy) = x², accum s2  (scalar engine)
                        xsq = work.tile([128, D], f32, name=f"xsq{s}",
                                        tag=f"xsq{s}")
                        nc.scalar.activation(xsq[:], x[s][:], Act.Square,
                                             accum_out=s2[:, s:s + 1])
                    # stats on [128, 2]
                    mean = stp.tile([128, 2], f32, tag="mean")
                    nc.vector.tensor_scalar(mean[:], s1[:], 1.0 / D, None,
                                            Alu.mult)
                    ssm = stp.tile([128, 2], f32, tag="ssm")
                    nc.vector.tensor_tensor(ssm[:], s1[:], mean[:], Alu.mult)
                    var = stp.tile([128, 2], f32, tag="var")
                    nc.vector.tensor_tensor(var[:], s2[:], ssm[:],
                                            Alu.subtract)
                    std = stp.tile([128, 2], f32, tag="std")
                    nc.scalar.activation(std[:], var[:], Act.Sqrt,
                                         scale=1.0 / (D - 1))
                    stde = stp.tile([128, 2], f32, tag="stde")
                    nc.vector.tensor_scalar(stde[:], std[:], EPS, None,
                                            Alu.add)
                    rstd = stp.tile([128, 2], f32, tag="rstd")
                    nc.vector.reciprocal(rstd[:], stde[:])
                    if t + 1 < S:
                        romg = stp.tile([128, 2], f32, tag="romg")
                        nc.vector.tensor_tensor(
                            romg[:], OMGt[:, :, t + 1], rstd[:], Alu.mult)
                        romg_p = romg
                    mean_p, rstd_p = mean, rstd

            # final normalize: av = (x - m) * rstd, DMA out
            for s in range(2):
                avf = work.tile([128, D], f32, name=f"avf{s}", tag=f"avf{s}")
                nc.vector.tensor_scalar(
                    avf[:], x[s][:], mean_p[:, s:s + 1], rstd_p[:, s:s + 1],
                    Alu.subtract, Alu.mult)
                nc.sync.dma_start(out_d[s], avf[:])
    nc.finalize()
    return nc


def _make_E():
    E = np.zeros((BL, 2, 128), np.float32)
    for s in range(2):
        for h in range(2):
            E[2 * s + h, s, 64 * h:64 * (h + 1)] = 1.0 / QS
    return E


def _get_nc():
    if "nc" not in _CACHE:
        _CACHE["nc"] = _build_phase3_nc()
    return _CACHE["nc"]


def _pack_inputs(Z, G_all):
    """Z [S,B,D] f32, G_all [S,B,A] f32 -> per-core in_maps."""
    import ml_dtypes
    bf16 = ml_dtypes.bfloat16
    Zq = np.clip(np.rint(Z * QS), -127, 127).astype(np.int8)  # [S,B,D]
    E = _make_E().astype(bf16)
    in_maps = []
    for i in range(N_CORES):
        zc = np.ascontiguousarray(
            Zq[:, 4 * i:4 * i + 4, :].transpose(1, 0, 2))     # [BL,S,D]
        gc = G_all[:, 4 * i:4 * i + 4, :]                     # [S,4,A]
        # Gd[p, s, t] = g[t, 2s + p//64, p%64]
        gd = np.ascontiguousarray(
            gc.reshape(S_ENC, 2, 2, A).transpose(2, 3, 1, 0)  # [h,a,s,t]
        ).reshape(128, 2, S_ENC).astype(bf16)
        in_maps.append({"Z": zc, "G": gd, "E": E})
    return in_maps


def _unpack_av(results):
    av = np.empty((B, A, D), np.float32)
    for i in range(N_CORES):
        o = results[i]["avout"]          # [2, 128, D]
        for s in range(2):
            for h in range(2):
                av[4 * i + 2 * s + h] = o[s, 64 * h:64 * (h + 1), :]
    return av


def _phase3_on_trn(Z, G_all):
    from concourse.bass_utils import run_bass_kernel_spmd
    nc = _get_nc()
    in_maps = _pack_inputs(Z, G_all)
    res = run_bass_kernel_spmd(nc, in_maps, core_ids=list(range(N_CORES)))
    return _unpack_av(res.results)


def _phase3_host(Z, G_all, n1_g, n1_b):
    """Fallback: vectorized numpy recurrence."""
    f = np.float32
    av = np.zeros((B, A, D), f)
    X = np.empty((B, A, D), f)
    for t in range(S_ENC):
        g = G_all[t][:, :, None]
        np.subtract(Z[t][:, None, :], av, out=X)
        X *= g
        av += X
        m = av.mean(-1, keepdims=True)
        av -= m
        q = np.einsum('bad,bad->ba', av, av)
        s = np.sqrt(q / (D - 1)) + EPS
        av /= s[:, :, None]
        if n1_g is not None:
            av *= n1_g
            av += n1_b
    return av


# --------------------------------------------------------------- model ----
def kernel(input_sequence, output_sequence, emb_in, emb_out, enc_key_W,
           enc_Wq, enc_bq, enc_Wk, enc_bk, n1_g, n1_b, dec_key_W,
           rdr_Wq, rdr_bq, rdr_Wk, rdr_bk, rdr_Wv, rdr_bv,
           dat_Wq, dat_bq, dat_Wk, dat_bk, n2_g, n2_b, n3_g, n3_b,
           voc_W, voc_b):
    f = np.float32
    emb_in = np.asarray(emb_in, f)
    scale = f(1.0 / math.sqrt(D))
    sqrtD = f(math.sqrt(D))
    idx = np.asarray(input_sequence)
    n1_g = np.asarray(n1_g, f)
    n1_b = np.asarray(n1_b, f)

    # -- encoder phase 1: z-trajectory (independent of av) --
    x_enc = emb_in[idx] * sqrtD                                # [B,S,D]
    Z = np.empty((S_ENC, B, D), f)
    z = np.zeros((B, D), f)
    for t in range(S_ENC):
        z = z + x_enc[:, t]
        m = z.mean(-1, keepdims=True)
        z -= m
        q = np.einsum('bd,bd->b', z, z)
        sd = np.sqrt(q / (D - 1)) + EPS
        z /= sd[:, None]
        if n1_g is not None:
            z *= n1_g
            z += n1_b
        Z[t] = z

    # -- encoder phase 2: batched gates (collapsed GEMM) --
    Qa = enc_key_W @ enc_Wq.T + enc_bq                         # [A,D]
    W2 = (enc_Wk.T @ Qa.T).astype(f)                           # [D,A]
    c2 = (enc_bk @ Qa.T).astype(f)                             # [A]
    G_all = _sigmoid((Z.reshape(-1, D) @ W2 + c2) * scale).reshape(
        S_ENC, B, A)

    # -- encoder phase 3: anchor-value recurrence on the NeuronCores --
    # device kernel computes plain LN; apply affine n1_g/n1_b after if
    # they are not identity (setup uses ones/zeros).
    affine = not (np.allclose(n1_g, 1.0) and np.allclose(n1_b, 0.0))
    if affine:
        av = _phase3_host(Z, G_all, n1_g, n1_b)
    else:
        try:
            av = _phase3_on_trn(Z, G_all)
        except Exception:
            av = _phase3_host(Z, G_all, None, None)

    # -- decoder (avx carry is dead code; z path only) --
    Kr = av @ rdr_Wk.T + rdr_bk                                # [B,A,D]
    Vr = av @ rdr_Wv.T + rdr_bv
    # fold the q-projection into the attention: s = zd @ M[b] + c[b]
    M = np.einsum('ed,bae->bda', np.asarray(rdr_Wq, f), Kr)    # [B,D,A]
    c = np.einsum('e,bae->ba', np.asarray(rdr_bq, f), Kr)      # [B,A]
    n2_g = np.asarray(n2_g, f)
    n2_b = np.asarray(n2_b, f)
    zd = Z[-1].copy()                                          # [B,D]
    for t in range(S_DEC):
        a = (np.einsum('bd,bda->ba', zd, M) + c) * scale       # [B,A]
        a -= a.max(axis=-1, keepdims=True)
        e = np.exp(a)
        e /= e.sum(axis=-1, keepdims=True)
        zd = zd + np.einsum('ba,bad->bd', e, Vr)
        m = zd.mean(-1, keepdims=True)
        zd -= m
        q = np.einsum('bd,bd->b', zd, zd)
        sd = np.sqrt(q / (D - 1)) + EPS
        zd /= sd[:, None]
        zd *= n2_g
        zd += n2_b

    # -- logits + log_softmax on host --
    zfin = zd.astype(f)                                        # [B,D]
    logits = zfin @ np.asarray(voc_W, f).T + voc_b             # [B,V]
    logits = logits[:, None, :]
    mx = logits.max(axis=-1, keepdims=True)
    lse = np.log(np.exp(logits - mx).sum(axis=-1, keepdims=True)) + mx
    return (logits - lse).astype(f)


# ------------------------------------------------------------- profile ----
def _profile():
    """Best-available timing of the bass kernel: HW NTFF if possible,
    else CoreSim cost-model time. Returns (exec_ns, source)."""
    nc = _get_nc()
    rng = np.random.default_rng(0)
    Z = rng.standard_normal((S_ENC, B, D)).astype(np.float32)
    G = (0.5 + 0.01 * rng.standard_normal((S_ENC, B, A))).astype(np.float32)
    in_maps = _pack_inputs(Z, G)
    try:
        from concourse.bass_utils import run_bass_kernel_spmd
        res = run_bass_kernel_spmd(nc, in_maps,
                                   core_ids=list(range(N_CORES)), trace=True)
        if res.exec_time_ns:
            return res.exec_time_ns, "hw-ntff"
    except Exception:
        pass
    from concourse.bass_interp import CoreSim
    sim = CoreSim(nc, publish_trace=False)
    for k, v in in_maps[0].items():
        sim.tensor(k)[:] = v
    sim.simulate()
    return int(sim.time), "coresim"
